# revision 1
# baseline (speedup 1.0000x reference)
"""Trainium2 Bass kernel for nn_ConditionalDisCoLoss.

loss = BCEWithLogits(inputs, targets)
     + dCor_masked(sigmoid(inputs), spectators, mask=spectators>=0.5)

Reformulation (no centered n x n matrices):
  p = sigmoid(x), m = (s >= 0.5), c = max(sum m, 1)
  A_i = sum_j m_i m_j |p_i - p_j|,  B_i likewise for s
  Sxy = sum_ij m_i m_j |p_i-p_j||s_i-s_j|
  Sxx = 2c*sum(m p^2) - 2(sum m p)^2   (closed form), Syy likewise
  Vxy = Sxy - (2/c) sum A_i B_i + (sum A)(sum B)/c^2  (and Vxx, Vyy)
  dcor = sqrt(max(Vxy,eps'))/sqrt(...)   with the reference's eps placement

Distribution + symmetry: the pair matrix is symmetric, so only j-bands
at or above each row's band are computed.  Global i-tiles (128 rows) are
dealt round-robin: core k owns i-tiles {8*it + k}, whose band is exactly
`it`, so every core runs the SAME program (jt in [it, 8)) on different
gathered row data - 36 of 64 tiles each.

Per tile [128 x 1024]:
 - PE: masked pairwise diffs D1 = m_i m_j (p_i - p_j) via K=4 bf16
   matmuls (bf16 hi+lo split of p keeps ~1e-7 element accuracy)
 - ACT: U = |D1| (bf16) + fused row-sum accum (A row-part); part of |D2|
 - DVE: rest of |D2| via abs_max + fused accum; product U*V with fused
   row-sum accum (Sxy partials)
 - PE: column sums of U,V for strictly-upper tiles (the transposed
   pairs' row sums) via [128,128]^T @ ones accumulated in one PSUM bank
Host combines per-core partial A/B vectors and scalars in float64.
"""

import numpy as np
from contextlib import ExitStack

import concourse.bass as bass
import concourse.bacc as bacc
import concourse.tile as tile
from concourse import mybir
from concourse.bass_utils import run_bass_kernel_spmd

N = 8192
NCORES = 8
STRIP = N // NCORES      # 1024 rows per core (gathered, not contiguous)
P = 128
JT = 1024                # j-tile width (one band = one j-tile)
NB = N // JT             # 8 bands
NIT = STRIP // P         # 8 i-tiles per core; i-tile it sits in band it
F_FULL = N // P          # 64
F_STRIP = STRIP // P     # 8
SPLIT_ACT = 704          # columns of |D2| done on ACT; rest on DVE

F32 = mybir.dt.float32
BF16 = mybir.dt.bfloat16
F32R = mybir.dt.float32r
ALU = mybir.AluOpType
ACTF = mybir.ActivationFunctionType
AX = mybir.AxisListType

NOUT = 16
# partials slots: 5 sum(R_diag), 6 sum(m), 7 sum(m*p), 8 sum(m*p^2),
#                 9 sum(m*s), 10 sum(m*s^2), 11 sum(bce), 12 sum(R_upper)
NCOLP = 112  # colparts: 7 bands x (8 quarters A | 8 quarters B)


def _build():
    nc = bacc.Bacc("TRN2", target_bir_lowering=False, debug=False,
                   num_devices=NCORES, enable_asserts=False)

    x_full = nc.dram_tensor("x_full", [N, 1], F32, kind="ExternalInput")
    s_full = nc.dram_tensor("s_full", [N], F32, kind="ExternalInput")
    x_strip = nc.dram_tensor("x_strip", [STRIP, 1], F32, kind="ExternalInput")
    t_strip = nc.dram_tensor("t_strip", [STRIP, 1], F32, kind="ExternalInput")
    s_strip = nc.dram_tensor("s_strip", [STRIP], F32, kind="ExternalInput")
    out = nc.dram_tensor("partials", [NOUT], F32, kind="ExternalOutput")
    rowp = nc.dram_tensor("rowparts", [P, 16], F32, kind="ExternalOutput")
    colp = nc.dram_tensor("colparts", [P, NCOLP], F32, kind="ExternalOutput")

    with tile.TileContext(nc) as tc, ExitStack() as ctx:
        pre = ctx.enter_context(tc.tile_pool(name="pre", bufs=1))
        uvp = ctx.enter_context(tc.tile_pool(name="uv", bufs=3))
        accp = ctx.enter_context(tc.tile_pool(name="acc", bufs=2))
        psp = ctx.enter_context(tc.tile_pool(name="psp", bufs=3, space="PSUM"))
        psc = ctx.enter_context(tc.tile_pool(name="psc", bufs=1, space="PSUM"))

        # ---------- preprocessing: full vectors -> moving operands ----------
        xf = pre.tile([P, F_FULL], F32)
        sf = pre.tile([P, F_FULL], F32)
        nc.sync.dma_start(out=xf, in_=x_full.ap().rearrange("(p f) one -> p (f one)", p=P))
        nc.scalar.dma_start(out=sf, in_=s_full.ap().rearrange("(p f) -> p f", p=P))

        pf = pre.tile([P, F_FULL], F32)
        nc.scalar.activation(pf, xf, ACTF.Sigmoid)
        mf = pre.tile([P, F_FULL], F32)
        nc.vector.tensor_scalar(mf, sf, 0.5, None, ALU.is_ge)
        af = pre.tile([P, F_FULL], F32)
        nc.vector.tensor_tensor(out=af, in0=mf, in1=pf, op=ALU.mult)
        cf = pre.tile([P, F_FULL], F32)
        nc.vector.tensor_tensor(out=cf, in0=mf, in1=sf, op=ALU.mult)

        # moving operands (f32, fed to the PE as float32r via bitcast):
        # RA rows: m, a=m*p   RB rows: m, c=m*s
        RA = pre.tile([2, N], F32)
        RB = pre.tile([2, N], F32)
        for eng, dst, row, src in ((nc.sync, RA, 0, mf), (nc.scalar, RA, 1, af),
                                   (nc.sync, RB, 0, mf), (nc.scalar, RB, 1, cf)):
            eng.dma_start(out=dst[row:row + 1, :], in_=src)

        # ---------- preprocessing: gathered strip -> stationary operands ----------
        # [16, 64] layout: strip position s = p*64 + f (DMA-friendly 256B rows)
        PS, FS = 16, 64
        xs = pre.tile([PS, FS], F32)
        ts = pre.tile([PS, FS], F32)
        ss = pre.tile([PS, FS], F32)
        nc.sync.dma_start(out=xs, in_=x_strip.ap().rearrange("(p f) one -> p (f one)", p=PS))
        nc.scalar.dma_start(out=ts, in_=t_strip.ap().rearrange("(p f) one -> p (f one)", p=PS))
        nc.sync.dma_start(out=ss, in_=s_strip.ap().rearrange("(p f) -> p f", p=PS))

        ps_ = pre.tile([PS, FS], F32)
        nc.scalar.activation(ps_, xs, ACTF.Sigmoid)
        ms = pre.tile([PS, FS], F32)
        nc.vector.tensor_scalar(ms, ss, 0.5, None, ALU.is_ge)
        negm = pre.tile([PS, FS], F32)
        nc.vector.tensor_scalar(negm, ms, -1.0, None, ALU.mult)

        bs = pre.tile([PS, FS], F32)
        nc.vector.tensor_tensor(out=bs, in0=ms, in1=ps_, op=ALU.mult)
        ds = pre.tile([PS, FS], F32)
        nc.vector.tensor_tensor(out=ds, in0=ms, in1=ss, op=ALU.mult)

        # stationary operands: LA rows (b, -m), LB rows (d, -m)
        LA = pre.tile([2, STRIP], F32)
        LB = pre.tile([2, STRIP], F32)
        for eng, dst, row, src in ((nc.sync, LA, 0, bs), (nc.scalar, LA, 1, negm),
                                   (nc.sync, LB, 0, ds), (nc.scalar, LB, 1, negm)):
            eng.dma_start(out=dst[row:row + 1, :], in_=src)

        # ---------- O(n) scalar columns (strip tiles live on partitions 0:16,
        # rest of cat stays zero and drops out of the final ones-matmul) ----------
        cat = pre.tile([P, NOUT], F32)
        nc.vector.memset(cat, 0.0)
        junk_s = pre.tile([PS, FS], F32)

        nc.vector.tensor_reduce(cat[0:PS, 6:7], ms, AX.X, ALU.add)
        nc.vector.tensor_reduce(cat[0:PS, 7:8], bs, AX.X, ALU.add)
        nc.vector.scalar_tensor_tensor(out=junk_s, in0=bs, scalar=0.0,
                                       in1=ps_, op0=ALU.bypass, op1=ALU.mult,
                                       accum_out=cat[0:PS, 8:9])
        nc.vector.tensor_reduce(cat[0:PS, 9:10], ds, AX.X, ALU.add)
        junk_s2 = pre.tile([PS, FS], F32)
        nc.vector.scalar_tensor_tensor(out=junk_s2, in0=ds, scalar=0.0,
                                       in1=ss, op0=ALU.bypass, op1=ALU.mult,
                                       accum_out=cat[0:PS, 10:11])

        # ---------- main pass: tiles (it, jt) with jt >= it ----------
        ones = pre.tile([P, 1], BF16)
        nc.vector.memset(ones, 1.0)
        onesf = pre.tile([P, 1], F32)
        nc.vector.memset(onesf, 1.0)

        # per-tile column sums, rectangular [it][jt][16] layout (no PSUM
        # accumulation -- scheduler may reorder same-engine matmuls, so
        # every tile writes its own fresh column; reduced over it at the end)
        colacc = psc.tile([P, NIT, NB, 16], F32)

        AA = pre.tile([P, NIT], F32)
        BB = pre.tile([P, NIT], F32)
        RRd = pre.tile([P, NIT], F32)
        RRu = pre.tile([P, NIT], F32)
        nc.vector.memset(RRu, 0.0)

        for it in range(NIT):
            njt = NB - it
            Ap = accp.tile([P, NB], F32, tag="Ap")
            Bp = accp.tile([P, 2 * NB], F32, tag="Bp")
            Rp = accp.tile([P, NB], F32, tag="Rp")
            lA = LA[:, it * P:(it + 1) * P]
            lB = LB[:, it * P:(it + 1) * P]
            for jj in range(njt):
                jt = it + jj
                psA = psp.tile([P, JT], F32, tag="ps")
                psB = psp.tile([P, JT], F32, tag="ps")
                for h in range(JT // 512):
                    j0 = jt * JT + h * 512
                    nc.tensor.matmul(psA[:, h * 512:(h + 1) * 512],
                                     lhsT=lA.bitcast(F32R),
                                     rhs=RA[:, j0:j0 + 512].bitcast(F32R),
                                     start=True, stop=True)
                    nc.tensor.matmul(psB[:, h * 512:(h + 1) * 512],
                                     lhsT=lB.bitcast(F32R),
                                     rhs=RB[:, j0:j0 + 512].bitcast(F32R),
                                     start=True, stop=True)
                U = uvp.tile([P, JT], BF16, tag="U")
                V = uvp.tile([P, JT], F32, tag="V")
                nc.scalar.activation(U, psA, ACTF.Abs, accum_out=Ap[:, jj:jj + 1])
                nc.scalar.activation(V[:, 0:SPLIT_ACT], psB[:, 0:SPLIT_ACT], ACTF.Abs,
                                     accum_out=Bp[:, 2 * jj:2 * jj + 1])
                # |x| on DVE in 2 ops (only one PSUM operand allowed per op):
                # Vn = -psB_slice (PSUM->SBUF), then V2 = max(Vn, psB_slice)
                Vn = uvp.tile([P, JT - SPLIT_ACT], F32, tag="Vn")
                nc.vector.tensor_scalar(Vn, psB[:, SPLIT_ACT:JT], -1.0, None, ALU.mult)
                nc.vector.scalar_tensor_tensor(out=V[:, SPLIT_ACT:JT],
                                               in0=Vn, scalar=0.0,
                                               in1=psB[:, SPLIT_ACT:JT],
                                               op0=ALU.bypass, op1=ALU.max,
                                               accum_out=Bp[:, 2 * jj + 1:2 * jj + 2])
                W = uvp.tile([P, JT], F32, tag="W")
                nc.vector.scalar_tensor_tensor(out=W, in0=U, scalar=0.0,
                                               in1=V, op0=ALU.bypass, op1=ALU.mult,
                                               accum_out=Rp[:, jj:jj + 1])
                if jt > it:
                    # transposed pairs' row sums = column sums, via PE
                    for q in range(8):
                        nc.tensor.matmul(colacc[:, it, jt, q:q + 1],
                                         lhsT=U[:, q * P:(q + 1) * P], rhs=ones,
                                         start=True, stop=True)
                        nc.tensor.matmul(colacc[:, it, jt, q + 8:q + 9],
                                         lhsT=V[:, q * P:(q + 1) * P], rhs=onesf,
                                         start=True, stop=True)
            nc.vector.tensor_reduce(AA[:, it:it + 1], Ap[:, 0:njt], AX.X, ALU.add)
            nc.vector.tensor_reduce(BB[:, it:it + 1], Bp[:, 0:2 * njt], AX.X, ALU.add)
            nc.vector.tensor_copy(RRd[:, it:it + 1], Rp[:, 0:1])
            if njt > 1:
                nc.vector.tensor_reduce(RRu[:, it:it + 1], Rp[:, 1:njt], AX.X, ALU.add)

        # ---------- outputs ----------
        # BCE partial: relu(x) - x*t + softplus(-|x|) = relu - xt + ln(1+exp(-|x|))
        rx = pre.tile([PS, FS], F32)
        nc.vector.tensor_scalar(rx, xs, 0.0, None, ALU.max)
        xt = pre.tile([PS, FS], F32)
        nc.vector.tensor_tensor(out=xt, in0=xs, in1=ts, op=ALU.mult)
        axx = pre.tile([PS, FS], F32)
        nc.scalar.activation(axx, xs, ACTF.Abs)
        enx = pre.tile([PS, FS], F32)
        nc.scalar.activation(enx, axx, ACTF.Exp, scale=-1.0)
        sp = pre.tile([PS, FS], F32)
        nc.scalar.activation(sp, enx, ACTF.Ln, bias=1.0)
        t1 = pre.tile([PS, FS], F32)
        nc.vector.tensor_tensor(out=t1, in0=rx, in1=xt, op=ALU.subtract)
        t2 = pre.tile([PS, FS], F32)
        nc.vector.scalar_tensor_tensor(out=t2, in0=t1, scalar=0.0, in1=sp,
                                       op0=ALU.add, op1=ALU.add,
                                       accum_out=cat[0:PS, 11:12])

        nc.vector.tensor_reduce(cat[:, 5:6], RRd, AX.X, ALU.add)
        nc.vector.tensor_reduce(cat[:, 12:13], RRu, AX.X, ALU.add)

        pcat = psp.tile([NOUT, 1], F32, tag="ps")
        nc.tensor.matmul(pcat, lhsT=cat, rhs=onesf, start=True, stop=True)
        outt = pre.tile([NOUT, 1], F32)
        nc.scalar.copy(outt, pcat)
        nc.sync.dma_start(out=out.ap().rearrange("(a b) -> a b", b=1), in_=outt)

        rowt = pre.tile([P, 16], F32)
        nc.vector.tensor_copy(rowt[:, 0:8], AA)
        nc.vector.tensor_copy(rowt[:, 8:16], BB)
        nc.sync.dma_start(out=rowp.ap(), in_=rowt)

        # reduce per-tile column sums over it (strided AP: last dim = it)
        colt = pre.tile([P, NCOLP], F32)
        for jt in range(1, NB):
            for half in range(2):  # 0: A quarters, 1: B quarters
                src = colacc[:, 0:jt, jt, half * 8:(half + 1) * 8]
                src = src.rearrange("p i q -> p q i")
                nc.vector.tensor_reduce(
                    colt[:, (jt - 1) * 16 + half * 8:(jt - 1) * 16 + (half + 1) * 8],
                    src, AX.X, ALU.add)
        nc.scalar.dma_start(out=colp.ap(), in_=colt)

    nc.compile()
    return nc


_NC_CACHE = None


def _get_nc():
    global _NC_CACHE
    if _NC_CACHE is None:
        _NC_CACHE = _build()
    return _NC_CACHE


def _row_index(k):
    """Global row indices owned by core k (i-tiles 8*it + k)."""
    idx = []
    for it_ in range(NIT):
        t = 8 * it_ + k
        idx.append(np.arange(t * P, (t + 1) * P))
    return np.concatenate(idx)


def _make_in_maps(inputs, targets, spectators):
    x = np.ascontiguousarray(np.asarray(inputs, dtype=np.float32)).reshape(N, 1)
    t = np.ascontiguousarray(np.asarray(targets, dtype=np.float32)).reshape(N, 1)
    s = np.ascontiguousarray(np.asarray(spectators, dtype=np.float32)).reshape(N)
    in_maps = []
    for k in range(NCORES):
        idx = _row_index(k)
        in_maps.append({
            "x_full": x,
            "s_full": s,
            "x_strip": np.ascontiguousarray(x[idx]),
            "t_strip": np.ascontiguousarray(t[idx]),
            "s_strip": np.ascontiguousarray(s[idx]),
        })
    return in_maps


def _combine(results):
    """results: list of per-core dicts with partials/rowparts/colparts."""
    g = np.zeros(NOUT, np.float64)
    A = np.zeros(N, np.float64)
    B = np.zeros(N, np.float64)
    for k in range(NCORES):
        g += results[k]["partials"].astype(np.float64)
        rowpart = results[k]["rowparts"].astype(np.float64)  # [128, 16]
        idx = _row_index(k)
        A[idx] += rowpart[:, 0:8].T.reshape(-1)
        B[idx] += rowpart[:, 8:16].T.reshape(-1)
        colpart = results[k]["colparts"].astype(np.float64)  # [128, 7*16]
        cp = colpart.reshape(P, 7, 16)
        # col index (jt-1)*16 + q (A) / + 8 + q (B); j = jt*1024 + q*128 + p
        Ac = cp[:, :, 0:8].transpose(1, 2, 0).reshape(-1)   # [7*8*128] j-ordered
        Bc = cp[:, :, 8:16].transpose(1, 2, 0).reshape(-1)
        A[JT:] += Ac
        B[JT:] += Bc

    cnt, smp, smp2, sms, sms2, bce_sum = g[6], g[7], g[8], g[9], g[10], g[11]
    Sxy = g[5] + 2.0 * g[12]
    sAB = float(A @ B)
    sAA = float(A @ A)
    sBB = float(B @ B)
    Tx = float(A.sum())
    Ty = float(B.sum())

    bce = bce_sum / N
    c = max(cnt, 1.0)
    Sxx = 2.0 * c * smp2 - 2.0 * smp * smp
    Syy = 2.0 * c * sms2 - 2.0 * sms * sms
    Vxy = Sxy - (2.0 / c) * sAB + Tx * Ty / (c * c)
    Vxx = Sxx - (2.0 / c) * sAA + Tx * Tx / (c * c)
    Vyy = Syy - (2.0 / c) * sBB + Ty * Ty / (c * c)
    EPS = 1e-8
    dcov = np.sqrt(max(Vxy / (c * c), EPS))
    dvx = np.sqrt(max(Vxx / (c * c), EPS))
    dvy = np.sqrt(max(Vyy / (c * c), EPS))
    dcor = dcov / (dvx * dvy)
    loss = bce + (dcor if cnt > 0 else 0.0)
    return np.float32(loss)


def kernel(inputs, targets, spectators):
    nc = _get_nc()
    in_maps = _make_in_maps(inputs, targets, spectators)
    res = run_bass_kernel_spmd(nc, in_maps, list(range(NCORES)))
    return _combine(res.results)


if __name__ == "__main__":
    d = np.load("/root/problem/cached_io.npz")
    out = kernel(d["inputs"], d["targets"], d["spectators"])
    exp = float(d["expected"])
    rel = abs(float(out) - exp) / abs(exp)
    print(f"kernel: {float(out):.8f}  expected: {exp:.8f}  rel err: {rel:.3e}")



# revision 17
# speedup vs baseline: 3.0104x; 3.0104x over previous
"""Trainium2 Bass kernel for nn_ConditionalDisCoLoss.

loss = BCEWithLogits(inputs, targets)
     + dCor_masked(sigmoid(inputs), spectators, mask=spectators>=0.5)

Host/device split (the sharding hint's "small filtered 1-D vectors"):
  * Host filters samples by the mask (c ~= 4080 of 8192 survive) and pads
    to CPAD; the dCor pair matrices shrink from n^2 to c^2 (~4x less work).
  * Host computes every O(c log c) term exactly in float64: row sums
    A_i = sum_j |p_i-p_j| and B_i (sort + prefix sums), Sxx/Syy closed
    forms, Tx/Ty/sAB/sAA/sBB.
  * Device computes the only quadratic term Sxy = sum_ij m_i m_j
    |p_i-p_j||s_i-s_j| plus the (linear) BCE partial sums.

Device per tile [128 x 1024] (jt >= it bands, round-robin i-tiles so all
8 cores run identical 10-tile programs):
  PE : masked pairwise diffs D1 = m_i m_j (p_i-p_j), D2 likewise for s,
       via rank-2 f32r matmuls (2 PSUM banks each, 4 matmuls)
  ACT: U = |D1|  (f32, PSUM->SBUF)
  DVE: P = U * D2  (bf16 out; |U*D2| == |D1||D2|; only DVE can read PSUM)
  DVE+Pool: |P| with fused row-sum accum -> Sxy partial columns; the DVE
       slice runs in 4x perf mode (all-bf16 SBUF operands), the idle
       gpsimd engine absorbs the wide remainder
BCE runs at the end on ACT (softplus, same act table as Abs) + DVE.
Host sums the [128, ncol] partials in float64 and assembles the loss.
"""

import numpy as np
from contextlib import ExitStack

import concourse.bass as bass
import concourse.bacc as bacc
import concourse.tile as tile
from concourse import mybir
from concourse.bass_utils import run_bass_kernel_spmd

N = 8192
NCORES = 8
P = 128
JT = 1024
CPAD = 4096              # padded filtered size (c=4080 for the reference seed)
CPAD_BIG = 5120          # fallback variant if c > 4096
WD = 128                 # columns of the |P| pass done on DVE; rest on Pool

F32 = mybir.dt.float32
BF16 = mybir.dt.bfloat16
F32R = mybir.dt.float32r
ALU = mybir.AluOpType
ACTF = mybir.ActivationFunctionType
AX = mybir.AxisListType

BSH = N // NCORES        # 1024 BCE samples per core
BCOL = BSH // P          # 8


def _tile_weights(nb):
    """Per-tile Sxy weights: diagonal-band tile once, strictly-upper twice."""
    w = []
    for it in range(nb):
        for jj in range(nb - it):
            w.append(1.0 if jj == 0 else 2.0)
    return w


def _build(cpad):
    nb = cpad // JT          # bands == i-tiles per core
    rows = cpad // NCORES    # stationary rows per core
    ntiles = nb * (nb + 1) // 2
    ncol = 2 * ntiles + 3    # 2 Sxy cols per tile + relu/ln1pexp/x*t sums

    nc = bacc.Bacc("TRN2", target_bir_lowering=False, debug=False,
                   num_devices=NCORES, enable_asserts=False)

    ra = nc.dram_tensor("ra", [2, cpad], F32R, kind="ExternalInput")
    rb = nc.dram_tensor("rb", [2, cpad], F32R, kind="ExternalInput")
    la = nc.dram_tensor("la", [2, rows], F32R, kind="ExternalInput")
    lb = nc.dram_tensor("lb", [2, rows], F32R, kind="ExternalInput")
    xs = nc.dram_tensor("xs", [P, BCOL], F32, kind="ExternalInput")
    ts = nc.dram_tensor("ts", [P, BCOL], F32, kind="ExternalInput")
    out = nc.dram_tensor("acc", [P, ncol], F32, kind="ExternalOutput")

    with tile.TileContext(nc) as tc, ExitStack() as ctx:
        pre = ctx.enter_context(tc.tile_pool(name="pre", bufs=1))
        uvp = ctx.enter_context(tc.tile_pool(name="uv", bufs=3))
        psp = ctx.enter_context(tc.tile_pool(name="psp", bufs=2, space="PSUM"))

        rat = pre.tile([2, cpad], F32R)
        rbt = pre.tile([2, cpad], F32R)
        lat = pre.tile([2, rows], F32R)
        lbt = pre.tile([2, rows], F32R)
        xst = pre.tile([P, BCOL], F32)
        tst = pre.tile([P, BCOL], F32)
        nc.sync.dma_start(out=rat, in_=ra.ap())
        nc.scalar.dma_start(out=rbt, in_=rb.ap())
        nc.sync.dma_start(out=lat, in_=la.ap())
        nc.scalar.dma_start(out=lbt, in_=lb.ap())
        nc.sync.dma_start(out=xst, in_=xs.ap())
        nc.scalar.dma_start(out=tst, in_=ts.ap())

        racc = pre.tile([P, ncol], F32)

        tix = 0
        for it in range(nb):
            lA = lat[:, it * P:(it + 1) * P]
            lB = lbt[:, it * P:(it + 1) * P]
            for jj in range(nb - it):
                jt = it + jj
                psA = psp.tile([P, JT], F32, tag="psA")
                psB = psp.tile([P, JT], F32, tag="psB")
                for h in range(JT // 512):
                    j0 = jt * JT + h * 512
                    nc.tensor.matmul(psA[:, h * 512:(h + 1) * 512],
                                     lhsT=lA, rhs=rat[:, j0:j0 + 512],
                                     start=True, stop=True)
                    nc.tensor.matmul(psB[:, h * 512:(h + 1) * 512],
                                     lhsT=lB, rhs=rbt[:, j0:j0 + 512],
                                     start=True, stop=True)
                U = uvp.tile([P, JT], F32, tag="U")
                nc.scalar.activation(U, psA, ACTF.Abs)
                # Signed product P = |D1| * D2 (only DVE can read PSUM), then
                # |P| = max(-P, P) in one stt per slice with fused row-sum
                # accum (Sxy partials). P is SBUF, so the idle gpsimd engine
                # absorbs the wide remainder of the abs pass.
                Pt = uvp.tile([P, JT], F32, tag="P")
                nc.vector.scalar_tensor_tensor(out=Pt, in0=U, scalar=0.0,
                                               in1=psB,
                                               op0=ALU.bypass, op1=ALU.mult)
                Wt = uvp.tile([P, JT], F32, tag="W")
                nc.vector.scalar_tensor_tensor(out=Wt[:, 0:WD],
                                               in0=Pt[:, 0:WD], scalar=-1.0,
                                               in1=Pt[:, 0:WD],
                                               op0=ALU.mult, op1=ALU.max,
                                               accum_out=racc[:, 2 * tix:2 * tix + 1])
                nc.vector.scalar_tensor_tensor(out=Wt[:, WD:JT],
                                               in0=Pt[:, WD:JT], scalar=-1.0,
                                               in1=Pt[:, WD:JT],
                                               op0=ALU.mult, op1=ALU.max,
                                               accum_out=racc[:, 2 * tix + 1:2 * tix + 2])
                tix += 1

        # BCE partials: softplus(x) = relu(x) + ln(1+exp(-|x|)), so
        # bce = (relu_sum + ln_sum - xt_sum)/N. Abs/Relu are in every act
        # table; Exp+Ln share natural_log_exp_and_others -> one reload max.
        rxj = pre.tile([P, BCOL], F32)
        nc.scalar.activation(rxj, xst, ACTF.Relu,
                             accum_out=racc[:, 2 * ntiles:2 * ntiles + 1])
        axj = pre.tile([P, BCOL], F32)
        nc.scalar.activation(axj, xst, ACTF.Abs)
        enj = pre.tile([P, BCOL], F32)
        nc.scalar.activation(enj, axj, ACTF.Exp, scale=-1.0)
        lnj = pre.tile([P, BCOL], F32)
        nc.scalar.activation(lnj, enj, ACTF.Ln, bias=1.0,
                             accum_out=racc[:, 2 * ntiles + 1:2 * ntiles + 2])
        xtj = pre.tile([P, BCOL], F32)
        nc.vector.scalar_tensor_tensor(out=xtj, in0=xst, scalar=0.0,
                                       in1=tst, op0=ALU.bypass, op1=ALU.mult,
                                       accum_out=racc[:, 2 * ntiles + 2:2 * ntiles + 3])

        nc.sync.dma_start(out=out.ap(), in_=racc)

    nc.compile()
    return nc


_NC_CACHE = {}


def _get_nc(cpad):
    if cpad not in _NC_CACHE:
        _NC_CACHE[cpad] = _build(cpad)
    return _NC_CACHE[cpad]


def _row_index(k, cpad):
    """Filtered-space row indices owned by core k (i-tiles NCORES*t + k)."""
    nit = cpad // JT
    idx = []
    for t in range(nit):
        g = NCORES * t + k
        idx.append(np.arange(g * P, (g + 1) * P))
    return np.concatenate(idx)


def _rowsums_abs(v):
    """A_i = sum_j |v_i - v_j| in O(c log c), exact float64."""
    o = np.argsort(v, kind="stable")
    q = v[o]
    pre = np.cumsum(q)
    tot = pre[-1]
    k = np.arange(len(q), dtype=np.float64)
    s = q * (2.0 * k + 2.0 - len(q)) - 2.0 * pre + tot
    a = np.empty_like(v)
    a[o] = s
    return a


def _numpy_loss(x, t, s):
    """Full-precision fallback (c > CPAD_BIG or degenerate inputs)."""
    x64 = x.astype(np.float64).reshape(-1)
    t64 = t.astype(np.float64).reshape(-1)
    s64 = s.astype(np.float64).reshape(-1)
    bce = np.mean(np.maximum(x64, 0) - x64 * t64 + np.log1p(np.exp(-np.abs(x64))))
    m = s.reshape(-1) >= np.float32(0.5)
    c = int(m.sum())
    if c == 0:
        return np.float32(bce)
    p = (1.0 / (1.0 + np.exp(-x64))).astype(np.float32)[m].astype(np.float64)
    ss = s.reshape(-1)[m].astype(np.float64)
    dx = np.abs(p[:, None] - p[None, :])
    dy = np.abs(ss[:, None] - ss[None, :])
    Sxy = float((dx * dy).sum())
    A = dx.sum(1); B = dy.sum(1)
    loss = bce + _assemble_dcor(c, Sxy, A, B, p, ss)
    return np.float32(loss)


def _assemble_dcor(c, Sxy, A, B, p, ss):
    Sxx = 2.0 * c * float((p * p).sum()) - 2.0 * float(p.sum()) ** 2
    Syy = 2.0 * c * float((ss * ss).sum()) - 2.0 * float(ss.sum()) ** 2
    Tx, Ty = float(A.sum()), float(B.sum())
    sAB, sAA, sBB = float(A @ B), float(A @ A), float(B @ B)
    Vxy = Sxy - 2.0 / c * sAB + Tx * Ty / c ** 2
    Vxx = Sxx - 2.0 / c * sAA + Tx * Tx / c ** 2
    Vyy = Syy - 2.0 / c * sBB + Ty * Ty / c ** 2
    EPS = 1e-8
    dcov = np.sqrt(max(Vxy / c ** 2, EPS))
    dvx = np.sqrt(max(Vxx / c ** 2, EPS))
    dvy = np.sqrt(max(Vyy / c ** 2, EPS))
    return dcov / (dvx * dvy)


def _prepare(inputs, targets, spectators):
    x = np.ascontiguousarray(np.asarray(inputs, dtype=np.float32)).reshape(N)
    t = np.ascontiguousarray(np.asarray(targets, dtype=np.float32)).reshape(N)
    s = np.ascontiguousarray(np.asarray(spectators, dtype=np.float32)).reshape(N)

    m = s >= np.float32(0.5)
    c = int(m.sum())
    cpad = CPAD if c <= CPAD else (CPAD_BIG if c <= CPAD_BIG else None)
    if cpad is None or c == 0:
        return None, None, None, (x, t, s)

    # p in f32, used consistently by host (A, Sxx) and device (Sxy)
    p32 = (1.0 / (1.0 + np.exp(-x.astype(np.float64)))).astype(np.float32)
    p_sel = p32[m]
    s_sel = s[m]

    p_pad = np.zeros(cpad, np.float32); p_pad[:c] = p_sel
    s_pad = np.zeros(cpad, np.float32); s_pad[:c] = s_sel
    m_pad = np.zeros(cpad, np.float32); m_pad[:c] = 1.0

    ra = np.ascontiguousarray(np.stack([m_pad, p_pad]))
    rb = np.ascontiguousarray(np.stack([m_pad, s_pad]))

    in_maps = []
    for k in range(NCORES):
        idx = _row_index(k, cpad)
        la = np.ascontiguousarray(np.stack([p_pad[idx], -m_pad[idx]]))
        lb = np.ascontiguousarray(np.stack([s_pad[idx], -m_pad[idx]]))
        xsk = np.ascontiguousarray(x[k * BSH:(k + 1) * BSH].reshape(P, BCOL))
        tsk = np.ascontiguousarray(t[k * BSH:(k + 1) * BSH].reshape(P, BCOL))
        in_maps.append({"ra": ra, "rb": rb, "la": la, "lb": lb,
                        "xs": xsk, "ts": tsk})

    meta = {
        "c": c, "cpad": cpad,
        "p_sel": p_sel.astype(np.float64),
        "s_sel": s_sel.astype(np.float64),
    }
    return cpad, in_maps, meta, None


def _combine(results, meta):
    cpad = meta["cpad"]
    nb = cpad // JT
    ntiles = nb * (nb + 1) // 2
    w = np.array(_tile_weights(nb), np.float64)

    Sxy = 0.0
    sp_sum = 0.0
    xt_sum = 0.0
    for res in results:
        cols = res["acc"].astype(np.float64).sum(axis=0)
        rt = cols[:2 * ntiles].reshape(ntiles, 2).sum(axis=1)
        Sxy += float(rt @ w)
        sp_sum += float(cols[2 * ntiles]) + float(cols[2 * ntiles + 1])
        xt_sum += float(cols[2 * ntiles + 2])

    bce = (sp_sum - xt_sum) / N
    c = meta["c"]
    p = meta["p_sel"]; ss = meta["s_sel"]
    A = _rowsums_abs(p); B = _rowsums_abs(ss)
    loss = bce + _assemble_dcor(c, Sxy, A, B, p, ss)
    return np.float32(loss)


def kernel(inputs, targets, spectators):
    cpad, in_maps, meta, fb = _prepare(inputs, targets, spectators)
    if fb is not None:
        return _numpy_loss(*fb)
    nc = _get_nc(cpad)
    res = run_bass_kernel_spmd(nc, in_maps, list(range(NCORES)))
    return _combine(res.results, meta)


if __name__ == "__main__":
    d = np.load("/root/problem/cached_io.npz")
    out = kernel(d["inputs"], d["targets"], d["spectators"])
    exp = float(d["expected"])
    rel = abs(float(out) - exp) / abs(exp)
    print(f"kernel: {float(out):.8f}  expected: {exp:.8f}  rel err: {rel:.3e}")


# revision 19
# speedup vs baseline: 3.4641x; 1.1507x over previous
"""Trainium2 Bass kernel for nn_ConditionalDisCoLoss.

loss = BCEWithLogits(inputs, targets)
     + dCor_masked(sigmoid(inputs), spectators, mask=spectators>=0.5)

Host/device split (the sharding hint's "small filtered 1-D vectors"):
  * Host filters samples by the mask (c ~= 4080 of 8192 survive) and pads
    to CPAD; the dCor pair matrices shrink from n^2 to c^2 (~4x less work).
  * Host computes every O(c log c) term exactly in float64: row sums
    A_i = sum_j |p_i-p_j| and B_i (sort + prefix sums), Sxx/Syy closed
    forms, Tx/Ty/sAB/sAA/sBB.
  * Device computes the only quadratic term Sxy = sum_ij m_i m_j
    |p_i-p_j||s_i-s_j| plus the (linear) BCE partial sums.

Device per tile [128 x 1024] (jt >= it bands, round-robin i-tiles so all
8 cores run identical 10-tile programs):
  PE : masked pairwise diffs D1 = m_i m_j (p_i-p_j), D2 likewise for s,
       via rank-2 f32r matmuls (2 PSUM banks each, 4 matmuls)
  ACT: U = |D1|  (f32, PSUM->SBUF)
  DVE: P = U * D2  (bf16 out; |U*D2| == |D1||D2|; only DVE can read PSUM)
  DVE+Pool: |P| with fused row-sum accum -> Sxy partial columns; the DVE
       slice runs in 4x perf mode (all-bf16 SBUF operands), the idle
       gpsimd engine absorbs the wide remainder
BCE runs at the end on ACT (softplus, same act table as Abs) + DVE.
Host sums the [128, ncol] partials in float64 and assembles the loss.
"""

import numpy as np
from contextlib import ExitStack

import concourse.bass as bass
import concourse.bacc as bacc
import concourse.tile as tile
from concourse import mybir
from concourse.bass_utils import run_bass_kernel_spmd

N = 8192
NCORES = 8
P = 128
JT = 1024
CPAD = 4096              # padded filtered size (c=4080 for the reference seed)
CPAD_BIG = 5120          # fallback variant if c > 4096
WD = 480                 # columns of the |P| pass done on ACT; rest on DVE

F32 = mybir.dt.float32
BF16 = mybir.dt.bfloat16
F32R = mybir.dt.float32r
ALU = mybir.AluOpType
ACTF = mybir.ActivationFunctionType
AX = mybir.AxisListType

BSH = N // NCORES        # 1024 BCE samples per core
BCOL = BSH // P          # 8


def _tile_weights(nb):
    """Per-tile Sxy weights: diagonal-band tile once, strictly-upper twice."""
    w = []
    for it in range(nb):
        for jj in range(nb - it):
            w.append(1.0 if jj == 0 else 2.0)
    return w


def _build(cpad):
    nb = cpad // JT          # bands == i-tiles per core
    rows = cpad // NCORES    # stationary rows per core
    ntiles = nb * (nb + 1) // 2
    ncol = 2 * ntiles + 3    # 2 Sxy cols per tile + relu/ln1pexp/x*t sums

    nc = bacc.Bacc("TRN2", target_bir_lowering=False, debug=False,
                   num_devices=NCORES, enable_asserts=False)

    ra = nc.dram_tensor("ra", [2, cpad], F32R, kind="ExternalInput")
    rb = nc.dram_tensor("rb", [2, cpad], F32R, kind="ExternalInput")
    la = nc.dram_tensor("la", [2, rows], F32R, kind="ExternalInput")
    lb = nc.dram_tensor("lb", [2, rows], F32R, kind="ExternalInput")
    xs = nc.dram_tensor("xs", [P, BCOL], F32, kind="ExternalInput")
    ts = nc.dram_tensor("ts", [P, BCOL], F32, kind="ExternalInput")
    out = nc.dram_tensor("acc", [P, ncol], F32, kind="ExternalOutput")

    with tile.TileContext(nc) as tc, ExitStack() as ctx:
        pre = ctx.enter_context(tc.tile_pool(name="pre", bufs=1))
        uvp = ctx.enter_context(tc.tile_pool(name="uv", bufs=3))
        psp = ctx.enter_context(tc.tile_pool(name="psp", bufs=2, space="PSUM"))

        rat = pre.tile([2, cpad], F32R)
        rbt = pre.tile([2, cpad], F32R)
        lat = pre.tile([2, rows], F32R)
        lbt = pre.tile([2, rows], F32R)
        xst = pre.tile([P, BCOL], F32)
        tst = pre.tile([P, BCOL], F32)
        nc.sync.dma_start(out=rat, in_=ra.ap())
        nc.scalar.dma_start(out=rbt, in_=rb.ap())
        nc.sync.dma_start(out=lat, in_=la.ap())
        nc.scalar.dma_start(out=lbt, in_=lb.ap())
        nc.sync.dma_start(out=xst, in_=xs.ap())
        nc.scalar.dma_start(out=tst, in_=ts.ap())

        racc = pre.tile([P, ncol], F32)

        tix = 0
        for it in range(nb):
            lA = lat[:, it * P:(it + 1) * P]
            lB = lbt[:, it * P:(it + 1) * P]
            for jj in range(nb - it):
                jt = it + jj
                psA = psp.tile([P, JT], F32, tag="psA")
                psB = psp.tile([P, JT], F32, tag="psB")
                for h in range(JT // 512):
                    j0 = jt * JT + h * 512
                    nc.tensor.matmul(psA[:, h * 512:(h + 1) * 512],
                                     lhsT=lA, rhs=rat[:, j0:j0 + 512],
                                     start=True, stop=True)
                    nc.tensor.matmul(psB[:, h * 512:(h + 1) * 512],
                                     lhsT=lB, rhs=rbt[:, j0:j0 + 512],
                                     start=True, stop=True)
                U = uvp.tile([P, JT], F32, tag="U")
                nc.scalar.activation(U, psA, ACTF.Abs)
                # Signed product P = |D1| * D2 (only DVE can read PSUM), then
                # |P| = max(-P, P) in one stt per slice with fused row-sum
                # accum (Sxy partials). P is SBUF, so the idle gpsimd engine
                # absorbs the wide remainder of the abs pass.
                Pt = uvp.tile([P, JT], F32, tag="P")
                nc.vector.scalar_tensor_tensor(out=Pt, in0=U, scalar=0.0,
                                               in1=psB,
                                               op0=ALU.bypass, op1=ALU.mult)
                # |P| with fused row-sum accum, split ACT (activation Abs,
                # which idles otherwise) / DVE (stt max(-P, P)).
                Wt = uvp.tile([P, JT], F32, tag="W")
                nc.scalar.activation(Wt[:, 0:WD], Pt[:, 0:WD], ACTF.Abs,
                                     accum_out=racc[:, 2 * tix:2 * tix + 1])
                nc.vector.scalar_tensor_tensor(out=Wt[:, WD:JT],
                                               in0=Pt[:, WD:JT], scalar=-1.0,
                                               in1=Pt[:, WD:JT],
                                               op0=ALU.mult, op1=ALU.max,
                                               accum_out=racc[:, 2 * tix + 1:2 * tix + 2])
                tix += 1

        # BCE partials: softplus(x) = relu(x) + ln(1+exp(-|x|)), so
        # bce = (relu_sum + ln_sum - xt_sum)/N. Abs/Relu are in every act
        # table; Exp+Ln share natural_log_exp_and_others -> one reload max.
        rxj = pre.tile([P, BCOL], F32)
        nc.scalar.activation(rxj, xst, ACTF.Relu,
                             accum_out=racc[:, 2 * ntiles:2 * ntiles + 1])
        axj = pre.tile([P, BCOL], F32)
        nc.scalar.activation(axj, xst, ACTF.Abs)
        enj = pre.tile([P, BCOL], F32)
        nc.scalar.activation(enj, axj, ACTF.Exp, scale=-1.0)
        lnj = pre.tile([P, BCOL], F32)
        nc.scalar.activation(lnj, enj, ACTF.Ln, bias=1.0,
                             accum_out=racc[:, 2 * ntiles + 1:2 * ntiles + 2])
        xtj = pre.tile([P, BCOL], F32)
        nc.vector.scalar_tensor_tensor(out=xtj, in0=xst, scalar=0.0,
                                       in1=tst, op0=ALU.bypass, op1=ALU.mult,
                                       accum_out=racc[:, 2 * ntiles + 2:2 * ntiles + 3])

        nc.sync.dma_start(out=out.ap(), in_=racc)

    nc.compile()
    return nc


_NC_CACHE = {}


def _get_nc(cpad):
    if cpad not in _NC_CACHE:
        _NC_CACHE[cpad] = _build(cpad)
    return _NC_CACHE[cpad]


def _row_index(k, cpad):
    """Filtered-space row indices owned by core k (i-tiles NCORES*t + k)."""
    nit = cpad // JT
    idx = []
    for t in range(nit):
        g = NCORES * t + k
        idx.append(np.arange(g * P, (g + 1) * P))
    return np.concatenate(idx)


def _rowsums_abs(v):
    """A_i = sum_j |v_i - v_j| in O(c log c), exact float64."""
    o = np.argsort(v, kind="stable")
    q = v[o]
    pre = np.cumsum(q)
    tot = pre[-1]
    k = np.arange(len(q), dtype=np.float64)
    s = q * (2.0 * k + 2.0 - len(q)) - 2.0 * pre + tot
    a = np.empty_like(v)
    a[o] = s
    return a


def _numpy_loss(x, t, s):
    """Full-precision fallback (c > CPAD_BIG or degenerate inputs)."""
    x64 = x.astype(np.float64).reshape(-1)
    t64 = t.astype(np.float64).reshape(-1)
    s64 = s.astype(np.float64).reshape(-1)
    bce = np.mean(np.maximum(x64, 0) - x64 * t64 + np.log1p(np.exp(-np.abs(x64))))
    m = s.reshape(-1) >= np.float32(0.5)
    c = int(m.sum())
    if c == 0:
        return np.float32(bce)
    p = (1.0 / (1.0 + np.exp(-x64))).astype(np.float32)[m].astype(np.float64)
    ss = s.reshape(-1)[m].astype(np.float64)
    dx = np.abs(p[:, None] - p[None, :])
    dy = np.abs(ss[:, None] - ss[None, :])
    Sxy = float((dx * dy).sum())
    A = dx.sum(1); B = dy.sum(1)
    loss = bce + _assemble_dcor(c, Sxy, A, B, p, ss)
    return np.float32(loss)


def _assemble_dcor(c, Sxy, A, B, p, ss):
    Sxx = 2.0 * c * float((p * p).sum()) - 2.0 * float(p.sum()) ** 2
    Syy = 2.0 * c * float((ss * ss).sum()) - 2.0 * float(ss.sum()) ** 2
    Tx, Ty = float(A.sum()), float(B.sum())
    sAB, sAA, sBB = float(A @ B), float(A @ A), float(B @ B)
    Vxy = Sxy - 2.0 / c * sAB + Tx * Ty / c ** 2
    Vxx = Sxx - 2.0 / c * sAA + Tx * Tx / c ** 2
    Vyy = Syy - 2.0 / c * sBB + Ty * Ty / c ** 2
    EPS = 1e-8
    dcov = np.sqrt(max(Vxy / c ** 2, EPS))
    dvx = np.sqrt(max(Vxx / c ** 2, EPS))
    dvy = np.sqrt(max(Vyy / c ** 2, EPS))
    return dcov / (dvx * dvy)


def _prepare(inputs, targets, spectators):
    x = np.ascontiguousarray(np.asarray(inputs, dtype=np.float32)).reshape(N)
    t = np.ascontiguousarray(np.asarray(targets, dtype=np.float32)).reshape(N)
    s = np.ascontiguousarray(np.asarray(spectators, dtype=np.float32)).reshape(N)

    m = s >= np.float32(0.5)
    c = int(m.sum())
    cpad = CPAD if c <= CPAD else (CPAD_BIG if c <= CPAD_BIG else None)
    if cpad is None or c == 0:
        return None, None, None, (x, t, s)

    # p in f32, used consistently by host (A, Sxx) and device (Sxy)
    p32 = (1.0 / (1.0 + np.exp(-x.astype(np.float64)))).astype(np.float32)
    p_sel = p32[m]
    s_sel = s[m]

    p_pad = np.zeros(cpad, np.float32); p_pad[:c] = p_sel
    s_pad = np.zeros(cpad, np.float32); s_pad[:c] = s_sel
    m_pad = np.zeros(cpad, np.float32); m_pad[:c] = 1.0

    ra = np.ascontiguousarray(np.stack([m_pad, p_pad]))
    rb = np.ascontiguousarray(np.stack([m_pad, s_pad]))

    in_maps = []
    for k in range(NCORES):
        idx = _row_index(k, cpad)
        la = np.ascontiguousarray(np.stack([p_pad[idx], -m_pad[idx]]))
        lb = np.ascontiguousarray(np.stack([s_pad[idx], -m_pad[idx]]))
        xsk = np.ascontiguousarray(x[k * BSH:(k + 1) * BSH].reshape(P, BCOL))
        tsk = np.ascontiguousarray(t[k * BSH:(k + 1) * BSH].reshape(P, BCOL))
        in_maps.append({"ra": ra, "rb": rb, "la": la, "lb": lb,
                        "xs": xsk, "ts": tsk})

    meta = {
        "c": c, "cpad": cpad,
        "p_sel": p_sel.astype(np.float64),
        "s_sel": s_sel.astype(np.float64),
    }
    return cpad, in_maps, meta, None


def _combine(results, meta):
    cpad = meta["cpad"]
    nb = cpad // JT
    ntiles = nb * (nb + 1) // 2
    w = np.array(_tile_weights(nb), np.float64)

    Sxy = 0.0
    sp_sum = 0.0
    xt_sum = 0.0
    for res in results:
        cols = res["acc"].astype(np.float64).sum(axis=0)
        rt = cols[:2 * ntiles].reshape(ntiles, 2).sum(axis=1)
        Sxy += float(rt @ w)
        sp_sum += float(cols[2 * ntiles]) + float(cols[2 * ntiles + 1])
        xt_sum += float(cols[2 * ntiles + 2])

    bce = (sp_sum - xt_sum) / N
    c = meta["c"]
    p = meta["p_sel"]; ss = meta["s_sel"]
    A = _rowsums_abs(p); B = _rowsums_abs(ss)
    loss = bce + _assemble_dcor(c, Sxy, A, B, p, ss)
    return np.float32(loss)


def kernel(inputs, targets, spectators):
    cpad, in_maps, meta, fb = _prepare(inputs, targets, spectators)
    if fb is not None:
        return _numpy_loss(*fb)
    nc = _get_nc(cpad)
    res = run_bass_kernel_spmd(nc, in_maps, list(range(NCORES)))
    return _combine(res.results, meta)


if __name__ == "__main__":
    d = np.load("/root/problem/cached_io.npz")
    out = kernel(d["inputs"], d["targets"], d["spectators"])
    exp = float(d["expected"])
    rel = abs(float(out) - exp) / abs(exp)
    print(f"kernel: {float(out):.8f}  expected: {exp:.8f}  rel err: {rel:.3e}")
